# revision 1
# baseline (speedup 1.0000x reference)
"""GAT (3-head, edge-weighted) message-passing kernel for 8 Trainium2 NeuronCores.

Strategy: sort edges by destination on host, give each core a contiguous
128-aligned destination-node range (49 windows x 128 nodes). Each core:
  phase 1: XT[n] = [x@W_lin+b | s_src(3) | s_dst(3)] (+ compact SDS[n,4] table)
  phase 2: per 128-edge tile: indirect-gather XT[src] and SDS[dst], softmax
           numerator p = exp(leakyrelu(s_src+s_dst)), one-hot scatter matmuls
           accumulate per-window denom [128,3] and agg [128f, 3*128n] in PSUM,
           per-window: project agg_h @ W_h scaled by 1/denom, write out rows.
No collectives needed: every core owns its dst range end-to-end.
"""

import numpy as np
import concourse.bass as bass
import concourse.bacc as bacc
import concourse.mybir as mybir
from concourse.tile import TileContext
from concourse import bass_utils

F32 = mybir.dt.float32
I32 = mybir.dt.int32

N_NODES = 50000
N_EDGES = 600000
DIM = 128
N_HEADS = 3
NEG_SLOPE = 0.2
NCORES = 8
NPW = 128                      # nodes per window
WPC = 49                       # windows per core
NPC = NPW * WPC                # 6272 nodes per core
NPAD = NPC * NCORES            # 50176
PADIDX = NPAD                  # poison row index (s_src = -1e4 -> p = 0)
NROWS = NPAD + 128             # 50304 rows in XT/SDS, 393 tiles of 128
NT1 = NROWS // 128             # phase-1 tiles

_cache = {}


def _phase1(nc, tc, cpool, xp, xt, sds, wg_sb, biasr_sb, poi_sb):
    with (
        tc.tile_pool(name="p1", bufs=6) as p1,
        tc.tile_pool(name="p1ps", bufs=4, space="PSUM") as p1ps,
    ):
        for i in range(NT1):
            xpt = p1.tile([128, DIM], F32, tag="xpt")
            nc.sync.dma_start(out=xpt[:], in_=xp[i * 128:(i + 1) * 128, :])
            ps = p1ps.tile([128, 138], F32, tag="ps1")
            nc.tensor.matmul(out=ps[:], lhsT=xpt[:], rhs=wg_sb[:],
                             start=True, stop=True)
            row = p1.tile([128, 138], F32, tag="row")
            nc.vector.tensor_tensor(out=row[:], in0=ps[:], in1=biasr_sb[:],
                                    op=mybir.AluOpType.add)
            nc.sync.dma_start(out=xt[i * 128:(i + 1) * 128, :],
                              in_=row[:, 0:134])
            nc.sync.dma_start(out=sds[i * 128:(i + 1) * 128, :],
                              in_=row[:, 134:138])
        # poison row: padding edges get s_src=-1e4 -> p = 0
        nc.sync.dma_start(out=xt[PADIDX:PADIDX + 1, 128:131], in_=poi_sb[:])


def _phase2(nc, tc, K, xt, sds, srca, wnid, dstc, ewa, ddia, outc,
            iota_sb, ident_sb, wh_sb, bias3_sb):
    with (
        tc.tile_pool(name="win", bufs=3) as wpool,
        tc.tile_pool(name="edge", bufs=16) as epool,
        tc.tile_pool(name="mst", bufs=6) as mpool,
        tc.tile_pool(name="oh", bufs=8) as ohpool,
        tc.tile_pool(name="sm", bufs=12) as smpool,
        tc.tile_pool(name="fl", bufs=3) as flpool,
        tc.tile_pool(name="aggps", bufs=2, space="PSUM") as aggps,
        tc.tile_pool(name="denps", bufs=2, space="PSUM") as denps,
        tc.tile_pool(name="sdps", bufs=2, space="PSUM") as sdps,
        tc.tile_pool(name="ops", bufs=2, space="PSUM") as ops,
    ):
        for w in range(WPC):
            r0 = w * 128
            srcw = wpool.tile([128, K], I32, tag="srcw")
            nc.sync.dma_start(out=srcw[:], in_=srca[r0:r0 + 128, :])
            dstcw = wpool.tile([128, K], F32, tag="dstcw")
            nc.sync.dma_start(out=dstcw[:], in_=dstc[r0:r0 + 128, :])
            eww = wpool.tile([128, K], F32, tag="eww")
            nc.sync.dma_start(out=eww[:], in_=ewa[r0:r0 + 128, :])
            ddiw = wpool.tile([128, K], F32, tag="ddiw")
            nc.sync.dma_start(out=ddiw[:], in_=ddia[r0:r0 + 128, :])
            wnw = wpool.tile([128, 1], I32, tag="wnw")
            nc.sync.dma_start(out=wnw[:], in_=wnid[r0:r0 + 128, :])
            sdw = wpool.tile([128, 4], F32, tag="sdw")
            nc.gpsimd.indirect_dma_start(
                out=sdw[:], out_offset=None, in_=sds[:],
                in_offset=bass.IndirectOffsetOnAxis(ap=wnw[:, 0:1], axis=0))
            ewn = wpool.tile([128, K], F32, tag="ewn")
            nc.vector.tensor_tensor(out=ewn[:], in0=eww[:], in1=ddiw[:],
                                    op=mybir.AluOpType.subtract)

            ps_den = denps.tile([128, 3], F32, tag="den")
            ps_agg = aggps.tile([128, N_HEADS * 128], F32, tag="agg")

            for t in range(K):
                xtg = epool.tile([128, 134], F32, tag="xtg")
                nc.gpsimd.indirect_dma_start(
                    out=xtg[:], out_offset=None, in_=xt[:],
                    in_offset=bass.IndirectOffsetOnAxis(
                        ap=srcw[:, t:t + 1], axis=0))
                onehot = ohpool.tile([128, 128], F32, tag="onehot")
                nc.vector.tensor_scalar(
                    out=onehot[:], in0=iota_sb[:],
                    scalar1=dstcw[:, t:t + 1], scalar2=None,
                    op0=mybir.AluOpType.is_equal)

                ps_mt = ops.tile([128, 128], F32, tag="pp")
                nc.tensor.transpose(out=ps_mt[:], in_=onehot[:],
                                    identity=ident_sb[:])
                mt = ohpool.tile([128, 128], F32, tag="mt")
                nc.scalar.activation(out=mt[:], in_=ps_mt[:],
                                     func=mybir.ActivationFunctionType.Copy)
                ps_sd = sdps.tile([128, 3], F32, tag="ps_sd")
                nc.tensor.matmul(out=ps_sd[:], lhsT=mt[:], rhs=sdw[:, 0:3],
                                 start=True, stop=True)
                e1 = smpool.tile([128, 3], F32, tag="e1")
                nc.vector.tensor_tensor(out=e1[:], in0=xtg[:, 128:131],
                                        in1=ps_sd[:],
                                        op=mybir.AluOpType.add)
                sc = smpool.tile([128, 3], F32, tag="sc")
                nc.vector.tensor_scalar(
                    out=sc[:], in0=e1[:], scalar1=NEG_SLOPE, scalar2=None,
                    op0=mybir.AluOpType.mult)
                t2 = smpool.tile([128, 3], F32, tag="t2")
                nc.vector.tensor_tensor(out=t2[:], in0=e1[:], in1=sc[:],
                                        op=mybir.AluOpType.max)
                p = smpool.tile([128, 3], F32, tag="p")
                nc.scalar.activation(out=p[:], in_=t2[:],
                                     func=mybir.ActivationFunctionType.Exp)

                nc.tensor.matmul(out=ps_den[:], lhsT=onehot[:], rhs=p[:],
                                 start=(t == 0), stop=(t == K - 1))

                q = smpool.tile([128, 3], F32, tag="q")
                nc.vector.tensor_scalar(
                    out=q[:], in0=p[:], scalar1=ewn[:, t:t + 1],
                    scalar2=None, op0=mybir.AluOpType.mult)

                mst = mpool.tile([128, N_HEADS * 128], F32, tag="mst")
                for h in range(2):
                    nc.vector.tensor_scalar(
                        out=mst[:, h * 128:(h + 1) * 128], in0=iota_sb[:],
                        scalar1=dstcw[:, t:t + 1],
                        scalar2=q[:, h:h + 1],
                        op0=mybir.AluOpType.is_equal,
                        op1=mybir.AluOpType.mult)
                # head 2 on ACT to offload DVE: onehot * q2
                nc.scalar.activation(out=mst[:, 256:384], in_=onehot[:],
                                     func=mybir.ActivationFunctionType.Copy,
                                     scale=q[:, 2:3])

                nc.tensor.matmul(out=ps_agg[:], lhsT=xtg[:, 0:128],
                                 rhs=mst[:], start=(t == 0),
                                 stop=(t == K - 1))

            # window flush
            den = flpool.tile([128, 3], F32, tag="dens")
            nc.vector.tensor_scalar(
                out=den[:], in0=ps_den[:], scalar1=1e-16, scalar2=3.0,
                op0=mybir.AluOpType.max, op1=mybir.AluOpType.mult)
            inv = flpool.tile([128, 3], F32, tag="inv")
            nc.vector.reciprocal(out=inv[:], in_=den[:])
            agg = flpool.tile([128, N_HEADS * 128], F32, tag="aggs")
            nc.scalar.activation(out=agg[:], in_=ps_agg[:],
                                 func=mybir.ActivationFunctionType.Copy)

            acc = flpool.tile([128, DIM], F32, tag="acc")
            tmp = flpool.tile([128, DIM], F32, tag="tmp")
            for h in range(N_HEADS):
                ps_o = ops.tile([128, DIM], F32, tag="pp")
                nc.tensor.matmul(out=ps_o[:],
                                 lhsT=agg[:, h * 128:(h + 1) * 128],
                                 rhs=wh_sb[:, h * DIM:(h + 1) * DIM],
                                 start=True, stop=True)
                dst_t = acc if h == 0 else tmp
                nc.vector.tensor_scalar(
                    out=dst_t[:], in0=ps_o[:], scalar1=inv[:, h:h + 1],
                    scalar2=None, op0=mybir.AluOpType.mult)
                if h > 0:
                    nc.vector.tensor_tensor(out=acc[:], in0=acc[:],
                                            in1=tmp[:],
                                            op=mybir.AluOpType.add)
            out_sb = flpool.tile([128, DIM], F32, tag="outsb")
            nc.vector.tensor_tensor(out=out_sb[:], in0=acc[:],
                                    in1=bias3_sb[:],
                                    op=mybir.AluOpType.add)
            nc.sync.dma_start(out=outc[r0:r0 + 128, :], in_=out_sb[:])


def _build(K):
    nc = bacc.Bacc("TRN2", target_bir_lowering=False, debug=False,
                   num_devices=NCORES)

    xp = nc.dram_tensor("xp", [NT1 * 128, DIM], F32, kind="ExternalInput")
    wg = nc.dram_tensor("wg", [DIM, 138], F32, kind="ExternalInput")
    biasr = nc.dram_tensor("biasr", [128, 138], F32, kind="ExternalInput")
    wheads = nc.dram_tensor("wheads", [N_HEADS * DIM, DIM], F32,
                            kind="ExternalInput")
    bias3 = nc.dram_tensor("bias3", [128, DIM], F32, kind="ExternalInput")
    iota = nc.dram_tensor("iota", [128, 128], F32, kind="ExternalInput")
    ident = nc.dram_tensor("ident", [128, 128], F32, kind="ExternalInput")
    poison = nc.dram_tensor("poison", [1, 3], F32, kind="ExternalInput")
    srca = nc.dram_tensor("srca", [WPC * 128, K], I32, kind="ExternalInput")
    wnid = nc.dram_tensor("wnid", [WPC * 128, 1], I32, kind="ExternalInput")
    dstg = nc.dram_tensor("dstg", [WPC * 128, K], I32, kind="ExternalInput")
    dstc = nc.dram_tensor("dstc", [WPC * 128, K], F32, kind="ExternalInput")
    ewa = nc.dram_tensor("ewa", [WPC * 128, K], F32, kind="ExternalInput")
    ddia = nc.dram_tensor("ddia", [WPC * 128, K], F32, kind="ExternalInput")

    xt = nc.dram_tensor("xt", [NROWS, 134], F32)
    sds = nc.dram_tensor("sds", [NROWS, 4], F32)
    outc = nc.dram_tensor("outc", [NPC, DIM], F32, kind="ExternalOutput")

    with TileContext(nc) as tc:
        with tc.tile_pool(name="const", bufs=1) as cpool:
            wg_sb = cpool.tile([DIM, 138], F32, tag="wg")
            nc.sync.dma_start(out=wg_sb[:], in_=wg[:])
            biasr_sb = cpool.tile([128, 138], F32, tag="biasr")
            nc.sync.dma_start(out=biasr_sb[:], in_=biasr[:])
            iota_sb = cpool.tile([128, 128], F32, tag="iota")
            nc.sync.dma_start(out=iota_sb[:], in_=iota[:])
            ident_sb = cpool.tile([128, 128], F32, tag="ident")
            nc.sync.dma_start(out=ident_sb[:], in_=ident[:])
            wh_sb = cpool.tile([128, N_HEADS * DIM], F32, tag="wh")
            for h in range(N_HEADS):
                nc.sync.dma_start(out=wh_sb[:, h * DIM:(h + 1) * DIM],
                                  in_=wheads[h * DIM:(h + 1) * DIM, :])
            bias3_sb = cpool.tile([128, DIM], F32, tag="bias3")
            nc.sync.dma_start(out=bias3_sb[:], in_=bias3[:])
            poi_sb = cpool.tile([1, 3], F32, tag="poi")
            nc.sync.dma_start(out=poi_sb[:], in_=poison[:])

            _phase1(nc, tc, cpool, xp, xt, sds, wg_sb, biasr_sb, poi_sb)
            _phase2(nc, tc, K, xt, sds, srca, wnid, dstc, ewa, ddia, outc,
                    iota_sb, ident_sb, wh_sb, bias3_sb)

    nc.compile()
    return nc


def _prep(x, edge_index, edge_ids, ddi_weight, W_lin, b_lin, edge_emb,
          W_heads, att_src, att_dst, bias_heads):
    x = np.asarray(x, np.float32)
    src = np.asarray(edge_index[0], np.int64)
    dst = np.asarray(edge_index[1], np.int64)
    eids = np.asarray(edge_ids, np.int64)
    ddi = np.asarray(ddi_weight, np.float32)
    W_lin = np.asarray(W_lin, np.float32)
    b_lin = np.asarray(b_lin, np.float32)
    edge_emb = np.asarray(edge_emb, np.float32)
    W_heads = np.asarray(W_heads, np.float32)
    att_src = np.asarray(att_src, np.float32)
    att_dst = np.asarray(att_dst, np.float32)
    bias_heads = np.asarray(bias_heads, np.float32)

    order = np.argsort(dst, kind="stable")
    src_s = src[order].astype(np.int32)
    dst_s = dst[order].astype(np.int32)
    ew0_s = edge_emb[eids[order], 0]
    ddi_s = ddi[order]

    bounds = np.searchsorted(dst_s, np.arange(0, NPAD + NPW, NPW))
    K = 1
    for c in range(NCORES):
        for w in range(WPC):
            wi = c * WPC + w
            K = max(K, (int(bounds[wi + 1] - bounds[wi]) + 127) // 128)

    per_core = []
    for c in range(NCORES):
        srca = np.full((WPC * 128, K), PADIDX, np.int32)
        dstga = np.zeros((WPC * 128, K), np.int32)
        dstca = np.zeros((WPC * 128, K), np.float32)
        ewa = np.zeros((WPC * 128, K), np.float32)
        ddia = np.zeros((WPC * 128, K), np.float32)
        for w in range(WPC):
            wi = c * WPC + w
            e0, e1 = int(bounds[wi]), int(bounds[wi + 1])
            n = e1 - e0
            base = wi * NPW
            dstga[w * 128:(w + 1) * 128, :] = base
            if n == 0:
                continue
            j = np.arange(n)
            pp = w * 128 + (j % 128)
            tt = j // 128
            srca[pp, tt] = src_s[e0:e1]
            dstga[pp, tt] = dst_s[e0:e1]
            dstca[pp, tt] = (dst_s[e0:e1] - base).astype(np.float32)
            ewa[pp, tt] = ew0_s[e0:e1]
            ddia[pp, tt] = ddi_s[e0:e1]
        wnid = (np.arange(WPC * 128, dtype=np.int32) + c * NPC)[:, None]
        per_core.append(dict(srca=srca, dstg=dstga, dstc=dstca,
                             ewa=ewa, ddia=ddia, wnid=wnid))

    # weight folding (host): scores s = x @ (W_lin@asd) + b@asd
    asd = np.zeros((DIM, 6), np.float32)
    for h in range(N_HEADS):
        asd[:, h] = W_heads[h] @ att_src[h]
        asd[:, 3 + h] = W_heads[h] @ att_dst[h]
    wg = np.zeros((DIM, 138), np.float32)
    wg[:, 0:128] = W_lin
    wg[:, 128:134] = W_lin @ asd
    wg[:, 134:137] = wg[:, 131:134]          # duplicate s_dst cols for SDS
    bias_ext = np.zeros(138, np.float32)
    bias_ext[0:128] = b_lin
    bias_ext[128:134] = b_lin @ asd
    bias_ext[134:137] = bias_ext[131:134]
    biasr = np.tile(bias_ext, (128, 1)).astype(np.float32)

    xpad = np.zeros((NT1 * 128, DIM), np.float32)
    xpad[:N_NODES] = x
    # phase-1 matmul lhsT must be x^T per 128-node tile
    xpt = np.zeros((NT1 * 128, DIM), np.float32)
    for i in range(NT1):
        xpt[i * 128:(i + 1) * 128] = xpad[i * 128:(i + 1) * 128].T
    wheads2 = W_heads.reshape(N_HEADS * DIM, DIM).copy()
    bias3 = np.tile(bias_heads.sum(0) / N_HEADS, (128, 1)).astype(np.float32)
    iota = np.tile(np.arange(128, dtype=np.float32), (128, 1))
    poisonv = np.full((1, 3), -1e4, np.float32)

    shared = dict(xp=xpt, wg=wg, biasr=biasr, wheads=wheads2, bias3=bias3,
                  iota=iota, ident=np.eye(128, dtype=np.float32),
                  poison=poisonv)
    in_maps = []
    for c in range(NCORES):
        m = dict(shared)
        m.update(per_core[c])
        in_maps.append(m)
    return K, in_maps


def kernel(**inputs):
    K, in_maps = _prep(**inputs)
    if K not in _cache:
        _cache[K] = _build(K)
    nc = _cache[K]
    res = bass_utils.run_bass_kernel_spmd(nc, in_maps,
                                          core_ids=list(range(NCORES)))
    out = np.concatenate([res.results[c]["outc"] for c in range(NCORES)],
                         axis=0)
    return np.ascontiguousarray(out[:N_NODES]).astype(np.float32)



# revision 15
# speedup vs baseline: 2.4991x; 2.4991x over previous
"""GAT (3-head, edge-weighted) message-passing kernel for 8 Trainium2 NeuronCores.

Strategy (edge-parallel, no collectives): nodes are packed into 392 windows of
128 (49 per core) with balanced in-edge counts. Each core owns its windows'
dst nodes end-to-end.
  phase 1: XT[r] = [x@W_lin | x@(W_lin@asd_src) | x@(W_lin@asd_dst)] rows in
           bf16 (512B stride) for two per-core compact node tables (<32768
           rows each, so dma_gather's int16 indices reach them), plus a
           contiguous per-window s_dst table.
  phase 2: per window: ONE batched dma_gather of all K*128 edge src rows;
           wide broadcast-compare builds of the transposed one-hot (ohT) and
           one-hot; s_dst per edge via ohT^T@sdw matmuls; p=exp(leakyrelu);
           denominator + ew-weighted denominator via one-hot matmuls; per-edge
           1/denominator via ohT@inv matmul so the three head projections,
           b_lin correction, and head bias all accumulate in one PSUM matmul
           chain. b_lin enters via the identity
             sum alpha*ew*(xW+b) @ Wh = (sum alpha*ew*xW) @ Wh + (sum alpha*ew) b@Wh.
"""

import numpy as np
import ml_dtypes
import concourse.bass as bass
import concourse.bacc as bacc
import concourse.mybir as mybir
from concourse.tile import TileContext
from concourse import bass_utils

F32 = mybir.dt.float32
BF16 = mybir.dt.bfloat16
I32 = mybir.dt.int32
I16 = mybir.dt.int16

N_NODES = 50000
N_EDGES = 600000
DIM = 128
N_HEADS = 3
NEG_SLOPE = 0.2
NCORES = 8
NPW = 128                      # nodes per window
WPC = 49                       # windows per core
NPC = NPW * WPC                # 6272 nodes per core
NWIN = NCORES * WPC            # 392 windows
WA = 25                        # windows in half A (per core)
WB = WPC - WA                  # 24 windows in half B

BF = ml_dtypes.bfloat16

_cache = {}


def _phase1(nc, tc, xq, xtab, sds, sds_row0, ntiles, own_tiles, wg_sb, biasr6_sb):
    """Project x for one half-table: XT rows + (for own tiles) sds rows."""
    B = 8
    with (
        tc.tile_pool(name="p1x", bufs=1) as p1x,
        tc.tile_pool(name="p1", bufs=3) as p1,
        tc.tile_pool(name="p1ps", bufs=2, space="PSUM") as p1ps,
    ):
        xq_sb = p1x.tile([128, ntiles * 128], BF16, tag="xq")
        nc.sync.dma_start(out=xq_sb[:], in_=xq[:])
        # batches: own tiles first (aligned), then the rest
        starts = []
        i = 0
        while i < own_tiles:
            b = min(B, own_tiles - i)
            starts.append((i, b, True))
            i += b
        while i < ntiles:
            b = min(B, ntiles - i)
            starts.append((i, b, False))
            i += b
        for (i0, b, own) in starts:
            ps = p1ps.tile([128, B * 128], F32, tag="ps")
            ps6 = p1ps.tile([128, B * 8], F32, tag="ps6")
            for j in range(b):
                nc.tensor.matmul(out=ps[:, j * 128:(j + 1) * 128],
                                 lhsT=xq_sb[:, (i0 + j) * 128:(i0 + j + 1) * 128],
                                 rhs=wg_sb[:, 0:128], start=True, stop=True)
                nc.tensor.matmul(out=ps6[:, j * 8:j * 8 + 6],
                                 lhsT=xq_sb[:, (i0 + j) * 128:(i0 + j + 1) * 128],
                                 rhs=wg_sb[:, 128:134], start=True, stop=True)
            row = p1.tile([128, B * 134], BF16, tag="row")
            r4 = row[:].rearrange("p (t c) -> p t c", t=B)
            nc.scalar.activation(
                out=r4[:, 0:b, 0:128],
                in_=ps[:, 0:b * 128].rearrange("p (t c) -> p t c", t=b),
                func=mybir.ActivationFunctionType.Copy)
            nc.vector.tensor_tensor(
                out=r4[:, 0:b, 128:134],
                in0=ps6[:].rearrange("p (t c) -> p t c", t=B)[:, 0:b, 0:6],
                in1=biasr6_sb[:].unsqueeze(1).broadcast_to([128, b, 6]),
                op=mybir.AluOpType.add)
            nc.sync.dma_start(
                out=xtab[i0 * 128:(i0 + b) * 128, 0:134].rearrange(
                    "(t p) c -> p t c", p=128),
                in_=row[:, 0:b * 134].rearrange("p (t c) -> p t c", t=b))
            if own:
                nc.sync.dma_start(
                    out=sds[sds_row0 + i0 * 128:sds_row0 + (i0 + b) * 128, :]
                    .rearrange("(t p) c -> p t c", p=128),
                    in_=row[:].rearrange("p (t c) -> p t c", t=B)[:, 0:b, 131:134])


def _phase2_half(nc, tc, K, half, nwin, wstart, xtab, earr, sds, outc,
                 iota_p_sb, iota_f_sb, ident_sb, wh_sb, bwh4_sb, pools):
    (wpool, gpool, bpool, spool, mpool, fpool,
     ps_small, ps_den, ps_agg, ps_proj) = pools
    KC = K * 128
    G = 4 * K                          # gidx i32 cols
    sdw_all = fpool.tile([128, nwin * 3], BF16, tag=f"sdw{half}")
    nc.sync.dma_start(
        out=sdw_all[:].rearrange("p (w c) -> p w c", w=nwin),
        in_=sds[wstart * 128:(wstart + nwin) * 128, :].rearrange(
            "(w p) c -> p w c", p=128))
    for wl in range(nwin):
        g0 = (wstart + wl) * 128
        ea = wpool.tile([128, 5 * K + 64 * K], I32, tag="ea")
        nc.sync.dma_start(out=ea[:], in_=earr[g0:g0 + 128, :])
        gidx = ea[:, 0:G].bitcast(I16)
        dstc = ea[:, G:G + K // 2].bitcast(BF16)
        ewa = ea[:, G + K // 2:5 * K].bitcast(BF16)
        dstb = ea[:, 5 * K:5 * K + 64 * K].bitcast(BF16)

        xgw = gpool.tile([128, K * 256], BF16, tag="xgw")
        # ucode caps one dma_gather at ~1024 descriptors; split into <=6-tile
        # chunks (768 idxs each)
        for c0 in range(0, K, 6):
            cw = min(6, K - c0)
            nc.gpsimd.dma_gather(
                out_ap=xgw[:].rearrange("p (t c) -> p t c", t=K)[:, c0:c0 + cw, :],
                in_ap=xtab[:], idxs_ap=gidx[:, c0 * 8:(c0 + cw) * 8],
                num_idxs=cw * 128, num_idxs_reg=cw * 128, elem_size=256)
        xg3 = xgw[:].rearrange("p (t c) -> p t c", t=K)

        oht = bpool.tile([128, KC], BF16, tag="oht")
        nc.vector.tensor_tensor(
            out=oht[:], in0=iota_p_sb[:].broadcast_to([128, KC]), in1=dstb,
            op=mybir.AluOpType.is_equal)
        ohw = bpool.tile([128, KC], BF16, tag="ohw")
        o3 = ohw[:].rearrange("p (t n) -> p t n", t=K)
        cc = 0
        while cc < K:
            cw = min(4, K - cc)
            nc.vector.tensor_tensor(
                out=o3[:, cc:cc + cw, :],
                in0=iota_f_sb[:].unsqueeze(1).broadcast_to([128, cw, 128]),
                in1=dstc[:, cc:cc + cw].unsqueeze(2).broadcast_to([128, cw, 128]),
                op=mybir.AluOpType.is_equal)
            cc += 4

        # per-edge s_dst, then p = exp(leakyrelu(s_src + s_dst))
        # psd cols 0:3K = s_dst per edge; cols 40:40+3K = inv_e per edge;
        # partitions 0:4 cols 80:208 = S1 transposed. All groups in this bank
        # open and close sequentially.
        psd = ps_small.tile([128, 208], F32, tag="psmall")
        iv0 = 3 * K + 4
        for t in range(K):
            nc.tensor.matmul(out=psd[:, 3 * t:3 * t + 3],
                             lhsT=oht[:, 128 * t:128 * (t + 1)],
                             rhs=sdw_all[:, 3 * wl:3 * wl + 3],
                             start=True, stop=True)
        e1 = spool.tile([128, 3 * K], F32, tag="e1")
        nc.vector.tensor_tensor(
            out=e1[:].rearrange("p (t c) -> p t c", t=K),
            in0=xg3[:, :, 128:131],
            in1=psd[:, 0:3 * K].rearrange("p (t c) -> p t c", t=K),
            op=mybir.AluOpType.add)
        sc = spool.tile([128, 3 * K], F32, tag="sc")
        nc.vector.tensor_scalar(out=sc[:], in0=e1[:], scalar1=NEG_SLOPE,
                                scalar2=None, op0=mybir.AluOpType.mult)
        t2 = spool.tile([128, 3 * K], F32, tag="t2")
        nc.vector.tensor_tensor(out=t2[:], in0=e1[:], in1=sc[:],
                                op=mybir.AluOpType.max)
        pq = spool.tile([128, 6 * K], BF16, tag="pq")
        q6 = pq[:].rearrange("p (t c) -> p t c", t=K)
        nc.scalar.activation(out=q6[:, :, 0:3],
                             in_=t2[:].rearrange("p (t c) -> p t c", t=K),
                             func=mybir.ActivationFunctionType.Exp)
        nc.vector.tensor_tensor(
            out=q6[:, :, 3:6], in0=q6[:, :, 0:3],
            in1=ewa[:].unsqueeze(2).broadcast_to([128, K, 3]),
            op=mybir.AluOpType.mult)

        # denominators: den[:,0:3]=sum p*onehot, den[:,3:6]=sum p*ew*onehot
        den = ps_den.tile([128, 6], F32, tag="den")
        for t in range(K):
            nc.tensor.matmul(out=den[:], lhsT=ohw[:, 128 * t:128 * (t + 1)],
                             rhs=pq[:, 6 * t:6 * t + 6],
                             start=(t == 0), stop=(t == K - 1))
        t3 = fpool.tile([128, 3], F32, tag="t3")
        nc.vector.tensor_scalar(out=t3[:], in0=den[:, 0:3],
                                scalar1=1e-16, scalar2=3.0,
                                op0=mybir.AluOpType.max, op1=mybir.AluOpType.mult)
        inv3 = fpool.tile([128, 3], BF16, tag="inv3")
        with nc.allow_low_precision(reason="softmax denom reciprocal in bf16"):
            nc.vector.reciprocal(out=inv3[:], in_=t3[:])
        s1 = fpool.tile([128, 4], BF16, tag="s1")
        nc.vector.memset(s1[:, 3:4], 1.0)
        nc.vector.tensor_tensor(out=s1[:, 0:3], in0=den[:, 3:6],
                                in1=inv3[:], op=mybir.AluOpType.mult)
        # per-edge 1/(3*den): inv_e = ohT^T @ inv3
        for t in range(K):
            nc.tensor.matmul(out=psd[:, iv0 + 3 * t:iv0 + 3 * t + 3],
                             lhsT=oht[:, 128 * t:128 * (t + 1)], rhs=inv3[:],
                             start=True, stop=True)
        qa2f = spool.tile([128, 3 * K], F32, tag="qa2f")
        nc.vector.tensor_tensor(
            out=qa2f[:].rearrange("p (t c) -> p t c", t=K),
            in0=q6[:, :, 3:6],
            in1=psd[:, iv0:iv0 + 3 * K].rearrange("p (t c) -> p t c", t=K),
            op=mybir.AluOpType.mult)
        qa2 = spool.tile([128, 3 * K], BF16, tag="qa2")
        nc.vector.tensor_scalar(out=qa2[:], in0=qa2f[:], scalar1=1.0,
                                scalar2=None, op0=mybir.AluOpType.mult)

        # scatter matrices mst[h] = onehot * qa2_h, heads split across engines
        mst = mpool.tile([128, 3 * KC], BF16, tag="mst")
        m4 = mst[:].rearrange("p (h t n) -> p h t n", h=3, t=K)
        cc = 0
        while cc < K:
            cw = min(4, K - cc)
            nc.vector.tensor_tensor(
                out=m4[:, 0, cc:cc + cw, :], in0=o3[:, cc:cc + cw, :],
                in1=qa2[:, 3 * cc:3 * (cc + cw):3].unsqueeze(2)
                .broadcast_to([128, cw, 128]),
                op=mybir.AluOpType.mult)
            nc.gpsimd.tensor_tensor(
                out=m4[:, 2, cc:cc + cw, :], in0=o3[:, cc:cc + cw, :],
                in1=qa2[:, 3 * cc + 2:3 * (cc + cw):3].unsqueeze(2)
                .broadcast_to([128, cw, 128]),
                op=mybir.AluOpType.mult)
            cc += 4
        for t in range(K):
            nc.scalar.activation(out=m4[:, 1, t, :],
                                 in_=ohw[:, 128 * t:128 * (t + 1)],
                                 func=mybir.ActivationFunctionType.Copy,
                                 scale=qa2f[:, 3 * t + 1:3 * t + 2])

        # aggregate: agg[f, (h,n)] += x_src[f] * mst
        agg = ps_agg.tile([128, 3 * 128], F32, tag="agg")
        for t in range(K):
            nc.tensor.matmul(out=agg[:], lhsT=xg3[:, t, 0:128],
                             rhs=m4[:, :, t, :],
                             start=(t == 0), stop=(t == K - 1))

        # flush: out = sum_h agg_h@Wh/(3 den) + S1@(b@Wh) + mean bias
        aggsb = fpool.tile([128, 3 * 128], BF16, tag="aggsb")
        nc.scalar.activation(out=aggsb[:], in_=agg[:],
                             func=mybir.ActivationFunctionType.Copy)
        s1t_ps = psd[0:4, 80:208].bitcast(BF16)[:, 0:128]
        nc.tensor.transpose(out=s1t_ps, in_=s1[:], identity=ident_sb[:])
        s1t = fpool.tile([4, 128], BF16, tag="s1ts")
        nc.scalar.activation(out=s1t[:], in_=s1t_ps,
                             func=mybir.ActivationFunctionType.Copy)
        proj = ps_proj.tile([128, 128], F32, tag="proj")
        for h in range(3):
            nc.tensor.matmul(out=proj[:], lhsT=aggsb[:, 128 * h:128 * (h + 1)],
                             rhs=wh_sb[:, 128 * h:128 * (h + 1)],
                             start=(h == 0), stop=False)
        nc.tensor.matmul(out=proj[:], lhsT=s1t[:], rhs=bwh4_sb[:],
                         start=False, stop=True)
        out_sb = fpool.tile([128, 128], F32, tag="outsb")
        nc.scalar.activation(out=out_sb[:], in_=proj[:],
                             func=mybir.ActivationFunctionType.Copy)
        nc.sync.dma_start(out=outc[g0:g0 + 128, :], in_=out_sb[:])


def _build(K, NTA, NTB):
    nc = bacc.Bacc("TRN2", target_bir_lowering=False, debug=False,
                   num_devices=NCORES)
    xqA = nc.dram_tensor("xqA", [128, NTA * 128], BF16, kind="ExternalInput")
    xqB = nc.dram_tensor("xqB", [128, NTB * 128], BF16, kind="ExternalInput")
    wg = nc.dram_tensor("wg", [128, 134], BF16, kind="ExternalInput")
    biasr6 = nc.dram_tensor("biasr6", [128, 6], BF16, kind="ExternalInput")
    wh = nc.dram_tensor("wh", [128, 3 * 128], BF16, kind="ExternalInput")
    bwh4 = nc.dram_tensor("bwh4", [4, 128], BF16, kind="ExternalInput")
    iota_p = nc.dram_tensor("iota_p", [128, 1], BF16, kind="ExternalInput")
    iota_f = nc.dram_tensor("iota_f", [128, 128], BF16, kind="ExternalInput")
    ident = nc.dram_tensor("ident", [128, 128], BF16, kind="ExternalInput")
    earr = nc.dram_tensor("earr", [WPC * 128, 69 * K], I32, kind="ExternalInput")

    xtabA = nc.dram_tensor("xtabA", [NTA * 128, 256], BF16)
    xtabB = nc.dram_tensor("xtabB", [NTB * 128, 256], BF16)
    sds = nc.dram_tensor("sds", [WPC * 128, 3], BF16)
    outc = nc.dram_tensor("outc", [NPC, DIM], F32, kind="ExternalOutput")

    with TileContext(nc) as tc:
        with tc.tile_pool(name="const", bufs=1) as cpool:
            wg_sb = cpool.tile([128, 134], BF16, tag="wg")
            nc.sync.dma_start(out=wg_sb[:], in_=wg[:])
            biasr6_sb = cpool.tile([128, 6], BF16, tag="biasr6")
            nc.sync.dma_start(out=biasr6_sb[:], in_=biasr6[:])
            wh_sb = cpool.tile([128, 3 * 128], BF16, tag="wh")
            nc.sync.dma_start(out=wh_sb[:], in_=wh[:])
            bwh4_sb = cpool.tile([4, 128], BF16, tag="bwh4")
            nc.sync.dma_start(out=bwh4_sb[:], in_=bwh4[:])
            iota_p_sb = cpool.tile([128, 1], BF16, tag="iota_p")
            nc.sync.dma_start(out=iota_p_sb[:], in_=iota_p[:])
            iota_f_sb = cpool.tile([128, 128], BF16, tag="iota_f")
            nc.sync.dma_start(out=iota_f_sb[:], in_=iota_f[:])
            ident_sb = cpool.tile([128, 128], BF16, tag="ident")
            nc.sync.dma_start(out=ident_sb[:], in_=ident[:])

            _phase1(nc, tc, xqA, xtabA, sds, 0, NTA, WA, wg_sb, biasr6_sb)
            _phase1(nc, tc, xqB, xtabB, sds, WA * 128, NTB, WB, wg_sb, biasr6_sb)

            with (
                tc.tile_pool(name="win", bufs=3) as wpool,
                tc.tile_pool(name="gat", bufs=2) as gpool,
                tc.tile_pool(name="big", bufs=2) as bpool,
                tc.tile_pool(name="sml", bufs=3) as spool,
                tc.tile_pool(name="mst", bufs=2) as mpool,
                tc.tile_pool(name="fl", bufs=3) as fpool,
                tc.tile_pool(name="psS", bufs=2, space="PSUM") as ps_small,
                tc.tile_pool(name="psD", bufs=2, space="PSUM") as ps_den,
                tc.tile_pool(name="psA", bufs=2, space="PSUM") as ps_agg,
                tc.tile_pool(name="psP", bufs=2, space="PSUM") as ps_proj,
            ):
                pools = (wpool, gpool, bpool, spool, mpool, fpool,
                         ps_small, ps_den, ps_agg, ps_proj)
                _phase2_half(nc, tc, K, 0, WA, 0, xtabA, earr, sds, outc,
                             iota_p_sb, iota_f_sb, ident_sb, wh_sb, bwh4_sb,
                             pools)
                _phase2_half(nc, tc, K, 1, WB, WA, xtabB, earr, sds, outc,
                             iota_p_sb, iota_f_sb, ident_sb, wh_sb, bwh4_sb,
                             pools)

    nc.compile()
    return nc


def _prep(x, edge_index, edge_ids, ddi_weight, W_lin, b_lin, edge_emb,
          W_heads, att_src, att_dst, bias_heads):
    x = np.asarray(x, np.float32)
    src = np.asarray(edge_index[0]).astype(np.int64)
    dst = np.asarray(edge_index[1]).astype(np.int64)
    eids = np.asarray(edge_ids).astype(np.int64)
    ddi = np.asarray(ddi_weight, np.float32)
    W_lin = np.asarray(W_lin, np.float32)
    b_lin = np.asarray(b_lin, np.float32)
    edge_emb = np.asarray(edge_emb, np.float32)
    W_heads = np.asarray(W_heads, np.float32)
    att_src = np.asarray(att_src, np.float32)
    att_dst = np.asarray(att_dst, np.float32)
    bias_heads = np.asarray(bias_heads, np.float32)
    ew = edge_emb[eids, 0] - ddi

    # --- balance nodes into NWIN windows of 128 nodes, equal edge counts ---
    import heapq
    deg = np.bincount(dst, minlength=N_NODES)
    order = np.argsort(-deg, kind="stable")
    heap = [(0, w) for w in range(NWIN)]
    heapq.heapify(heap)
    slots_used = np.zeros(NWIN, np.int32)
    loads = np.zeros(NWIN, np.int64)
    win_of = np.empty(N_NODES, np.int32)
    slot_of = np.empty(N_NODES, np.int32)
    for n in order:
        load, w = heapq.heappop(heap)
        win_of[n] = w
        slot_of[n] = slots_used[w]
        slots_used[w] += 1
        loads[w] += deg[n]
        if slots_used[w] < NPW:
            heapq.heappush(heap, (int(loads[w]), w))
    K = int((loads.max() + NPW - 1) // NPW)
    K += K % 2  # even, for bf16 pairs in the i32 container

    ewin = win_of[dst]                # window of each edge
    eorder = np.argsort(ewin, kind="stable")
    esrc = src[eorder]
    edst = dst[eorder]
    eew = ew[eorder]
    ewin_s = ewin[eorder]
    wbounds = np.searchsorted(ewin_s, np.arange(NWIN + 1))

    # --- per-core compact tables and edge arrays ---
    halves = [(0, WA), (WA, WB)]
    NTA = NTB = 0
    core_data = []
    for c in range(NCORES):
        hd = []
        for hi, (w0, nw) in enumerate(halves):
            gw0 = c * WPC + w0
            own_nodes = np.full(nw * 128, -1, np.int64)
            for wl in range(nw):
                wsel = np.where(win_of == gw0 + wl)[0]
                own_nodes[wl * 128 + slot_of[wsel]] = wsel
            e0, e1 = wbounds[gw0], wbounds[gw0 + nw]
            hsrc = esrc[e0:e1]
            own_set = own_nodes[own_nodes >= 0]
            relab = np.full(N_NODES, -1, np.int32)
            own_rows = np.where(own_nodes >= 0)[0]
            relab[own_nodes[own_rows]] = own_rows
            extra = np.unique(hsrc)
            extra = extra[relab[extra] < 0]
            base = nw * 128
            relab[extra] = base + np.arange(len(extra), dtype=np.int32)
            nrows = base + len(extra)
            assert nrows <= 32767, f"compact table too large: {nrows}"
            node_of_row = np.full(nrows, -1, np.int64)
            node_of_row[own_rows] = own_nodes[own_rows]
            node_of_row[base:] = extra
            hd.append(dict(w0=w0, nw=nw, gw0=gw0, e0=e0, e1=e1,
                           relab=relab, node_of_row=node_of_row, nrows=nrows))
        NTA = max(NTA, (hd[0]["nrows"] + 127) // 128)
        NTB = max(NTB, (hd[1]["nrows"] + 127) // 128)
        core_data.append(hd)

    # --- weights / consts ---
    asd = np.zeros((DIM, 6), np.float32)
    for h in range(N_HEADS):
        asd[:, h] = W_heads[h] @ att_src[h]
        asd[:, 3 + h] = W_heads[h] @ att_dst[h]
    wg = np.zeros((DIM, 134), np.float32)
    wg[:, 0:128] = W_lin
    wg[:, 128:134] = W_lin @ asd
    biasr6 = np.tile(b_lin @ asd, (128, 1))
    wh2 = np.zeros((128, 3 * 128), np.float32)
    for h in range(N_HEADS):
        wh2[:, h * 128:(h + 1) * 128] = W_heads[h]
    bwh4 = np.zeros((4, 128), np.float32)
    for h in range(N_HEADS):
        bwh4[h] = b_lin @ W_heads[h]
    bwh4[3] = bias_heads.mean(0)
    iota_p = np.arange(128, dtype=np.float32).reshape(128, 1)
    iota_f = np.tile(np.arange(128, dtype=np.float32), (128, 1))
    ident = np.eye(128, dtype=np.float32)
    shared = dict(wg=wg.astype(BF), biasr6=biasr6.astype(BF),
                  wh=wh2.astype(BF), bwh4=bwh4.astype(BF),
                  iota_p=iota_p.astype(BF), iota_f=iota_f.astype(BF),
                  ident=ident.astype(BF))

    in_maps = []
    for c in range(NCORES):
        m = dict(shared)
        earr = np.zeros((WPC * 128, 69 * K), np.int32)
        for hi, (w0, nw) in enumerate(halves):
            hdd = core_data[c][hi]
            nt = NTA if hi == 0 else NTB
            xq = np.zeros((128, nt * 128), BF)
            valid = hdd["node_of_row"] >= 0
            cols = np.where(valid)[0]
            xq[:, cols] = x[hdd["node_of_row"][cols]].T.astype(BF)
            m["xqA" if hi == 0 else "xqB"] = xq
            relab = hdd["relab"]
            for wl in range(nw):
                gw = hdd["gw0"] + wl
                e0, e1 = wbounds[gw], wbounds[gw + 1]
                mcount = e1 - e0
                # per-window K*128 edge slots, j -> (p=j%128, t=j//128)
                gi = np.zeros(K * 128, np.int16)
                gi[:mcount] = relab[esrc[e0:e1]].astype(np.int16)
                dc = np.full(K * 128, 128.0, np.float32)
                dc[:mcount] = slot_of[edst[e0:e1]].astype(np.float32)
                ewv = np.zeros(K * 128, np.float32)
                ewv[:mcount] = eew[e0:e1]
                g16 = np.zeros((16, K * 8), np.int16)
                g16[np.arange(K * 128) % 16, np.arange(K * 128) // 16] = gi
                row = (w0 + wl) * 128
                blk = earr[row:row + 128]
                blk[:, 0:4 * K] = np.tile(g16, (8, 1)).view(np.int32)
                dcol = np.ascontiguousarray(dc.reshape(K, 128).T).astype(BF)
                ecol = np.ascontiguousarray(ewv.reshape(K, 128).T).astype(BF)
                blk[:, 4 * K:4 * K + K // 2] = dcol.view(np.int32)
                blk[:, 4 * K + K // 2:5 * K] = ecol.view(np.int32)
                dstb = np.tile(dc.astype(BF), (128, 1))     # [128, K*128]
                blk[:, 5 * K:69 * K] = dstb.view(np.int32)
        m["earr"] = earr
        in_maps.append(m)

    key = (K, NTA, NTB)
    # map node -> global output row
    gslot = (win_of.astype(np.int64) // WPC) * NPC + \
        (win_of.astype(np.int64) % WPC) * 128 + slot_of
    return key, dict(in_maps=in_maps, gslot=gslot)


def kernel(**inputs):
    key, d = _prep(**inputs)
    if key not in _cache:
        _cache[key] = _build(*key)
    nc = _cache[key]
    res = bass_utils.run_bass_kernel_spmd(nc, d["in_maps"],
                                          core_ids=list(range(NCORES)))
    big = np.concatenate([res.results[c]["outc"] for c in range(NCORES)],
                         axis=0)
    out = big[d["gslot"]]
    return np.ascontiguousarray(out).astype(np.float32)


# revision 16
# speedup vs baseline: 2.9368x; 1.1752x over previous
"""GAT (3-head, edge-weighted) message-passing kernel for 8 Trainium2 NeuronCores.

Strategy (edge-parallel, no collectives): nodes are packed into 392 windows of
128 (49 per core) with balanced in-edge counts. Each core owns its windows'
dst nodes end-to-end.
  phase 1: XT[r] = [x@W_lin | x@(W_lin@asd_src) | x@(W_lin@asd_dst)] rows in
           bf16 (512B stride) for two per-core compact node tables (<32768
           rows each, so dma_gather's int16 indices reach them), plus a
           contiguous per-window s_dst table.
  phase 2: per window: ONE batched dma_gather of all K*128 edge src rows;
           wide broadcast-compare builds of the transposed one-hot (ohT) and
           one-hot; s_dst per edge via ohT^T@sdw matmuls; p=exp(leakyrelu);
           denominator + ew-weighted denominator via one-hot matmuls; per-edge
           1/denominator via ohT@inv matmul so the three head projections,
           b_lin correction, and head bias all accumulate in one PSUM matmul
           chain. b_lin enters via the identity
             sum alpha*ew*(xW+b) @ Wh = (sum alpha*ew*xW) @ Wh + (sum alpha*ew) b@Wh.
"""

import numpy as np
import ml_dtypes
import concourse.bass as bass
import concourse.bacc as bacc
import concourse.mybir as mybir
from concourse.tile import TileContext
from concourse import bass_utils

F32 = mybir.dt.float32
BF16 = mybir.dt.bfloat16
I32 = mybir.dt.int32
I16 = mybir.dt.int16

N_NODES = 50000
N_EDGES = 600000
DIM = 128
N_HEADS = 3
NEG_SLOPE = 0.2
NCORES = 8
NPW = 128                      # nodes per window
WPC = 49                       # windows per core
NPC = NPW * WPC                # 6272 nodes per core
NWIN = NCORES * WPC            # 392 windows
WA = 25                        # windows in half A (per core)
WB = WPC - WA                  # 24 windows in half B

BF = ml_dtypes.bfloat16

_cache = {}


def _phase1(nc, tc, xq, xtab, sds, sds_row0, ntiles, own_tiles, wg_sb, biasr6_sb):
    """Project x for one half-table: XT rows + (for own tiles) sds rows."""
    B = 8
    with (
        tc.tile_pool(name="p1x", bufs=1) as p1x,
        tc.tile_pool(name="p1", bufs=3) as p1,
        tc.tile_pool(name="p1ps", bufs=2, space="PSUM") as p1ps,
    ):
        xq_sb = p1x.tile([128, ntiles * 128], BF16, tag="xq")
        nc.sync.dma_start(out=xq_sb[:], in_=xq[:])
        # batches: own tiles first (aligned), then the rest
        starts = []
        i = 0
        while i < own_tiles:
            b = min(B, own_tiles - i)
            starts.append((i, b, True))
            i += b
        while i < ntiles:
            b = min(B, ntiles - i)
            starts.append((i, b, False))
            i += b
        for (i0, b, own) in starts:
            ps = p1ps.tile([128, B * 128], F32, tag="ps")
            ps6 = p1ps.tile([128, B * 8], F32, tag="ps6")
            for j in range(b):
                nc.tensor.matmul(out=ps[:, j * 128:(j + 1) * 128],
                                 lhsT=xq_sb[:, (i0 + j) * 128:(i0 + j + 1) * 128],
                                 rhs=wg_sb[:, 0:128], start=True, stop=True)
                nc.tensor.matmul(out=ps6[:, j * 8:j * 8 + 6],
                                 lhsT=xq_sb[:, (i0 + j) * 128:(i0 + j + 1) * 128],
                                 rhs=wg_sb[:, 128:134], start=True, stop=True)
            row = p1.tile([128, B * 134], BF16, tag="row")
            r4 = row[:].rearrange("p (t c) -> p t c", t=B)
            nc.scalar.activation(
                out=r4[:, 0:b, 0:128],
                in_=ps[:, 0:b * 128].rearrange("p (t c) -> p t c", t=b),
                func=mybir.ActivationFunctionType.Copy)
            nc.vector.tensor_tensor(
                out=r4[:, 0:b, 128:134],
                in0=ps6[:].rearrange("p (t c) -> p t c", t=B)[:, 0:b, 0:6],
                in1=biasr6_sb[:].unsqueeze(1).broadcast_to([128, b, 6]),
                op=mybir.AluOpType.add)
            nc.sync.dma_start(
                out=xtab[i0 * 128:(i0 + b) * 128, 0:134].rearrange(
                    "(t p) c -> p t c", p=128),
                in_=row[:, 0:b * 134].rearrange("p (t c) -> p t c", t=b))
            if own:
                nc.sync.dma_start(
                    out=sds[sds_row0 + i0 * 128:sds_row0 + (i0 + b) * 128, :]
                    .rearrange("(t p) c -> p t c", p=128),
                    in_=row[:].rearrange("p (t c) -> p t c", t=B)[:, 0:b, 131:134])


def _phase2_half(nc, tc, K, half, nwin, wstart, xtab, earr, sds, outc,
                 iota_p_sb, iota_f_sb, ident_sb, wh_sb, bwh4_sb, pools):
    (wpool, gpool, bpool, spool, mpool, fpool,
     ps_small, ps_agg, ps_proj) = pools
    KC = K * 128
    G = 4 * K                          # gidx i32 cols
    sdw_all = fpool.tile([128, nwin * 3], BF16, tag=f"sdw{half}")
    nc.sync.dma_start(
        out=sdw_all[:].rearrange("p (w c) -> p w c", w=nwin),
        in_=sds[wstart * 128:(wstart + nwin) * 128, :].rearrange(
            "(w p) c -> p w c", p=128))
    for wl in range(nwin):
        g0 = (wstart + wl) * 128
        ea = wpool.tile([128, 5 * K + 64 * K], I32, tag="ea")
        nc.sync.dma_start(out=ea[:], in_=earr[g0:g0 + 128, :])
        gidx = ea[:, 0:G].bitcast(I16)
        dstc = ea[:, G:G + K // 2].bitcast(BF16)
        ewa = ea[:, G + K // 2:5 * K].bitcast(BF16)
        dstb = ea[:, 5 * K:5 * K + 64 * K].bitcast(BF16)

        xgw = gpool.tile([128, K * 256], BF16, tag="xgw")
        # ucode caps one dma_gather at ~1024 descriptors; split into <=6-tile
        # chunks (768 idxs each)
        for c0 in range(0, K, 6):
            cw = min(6, K - c0)
            nc.gpsimd.dma_gather(
                out_ap=xgw[:].rearrange("p (t c) -> p t c", t=K)[:, c0:c0 + cw, :],
                in_ap=xtab[:], idxs_ap=gidx[:, c0 * 8:(c0 + cw) * 8],
                num_idxs=cw * 128, num_idxs_reg=cw * 128, elem_size=256)
        xg3 = xgw[:].rearrange("p (t c) -> p t c", t=K)

        oht = bpool.tile([128, KC], BF16, tag="oht")
        nc.vector.tensor_tensor(
            out=oht[:], in0=iota_p_sb[:].broadcast_to([128, KC]), in1=dstb,
            op=mybir.AluOpType.is_equal)
        ohw = bpool.tile([128, KC], BF16, tag="ohw")
        o3 = ohw[:].rearrange("p (t n) -> p t n", t=K)
        cc = 0
        while cc < K:
            cw = min(4, K - cc)
            nc.vector.tensor_tensor(
                out=o3[:, cc:cc + cw, :],
                in0=iota_f_sb[:].unsqueeze(1).broadcast_to([128, cw, 128]),
                in1=dstc[:, cc:cc + cw].unsqueeze(2).broadcast_to([128, cw, 128]),
                op=mybir.AluOpType.is_equal)
            cc += 4

        # per-edge s_dst, then p = exp(leakyrelu(s_src + s_dst))
        # psd cols 0:3K = s_dst per edge; cols 40:40+3K = inv_e per edge;
        # partitions 0:4 cols 80:208 = S1 transposed. All groups in this bank
        # open and close sequentially.
        psd = ps_small.tile([128, 512], F32, tag="psmall")
        iv0 = 3 * K + 52
        for t in range(K):
            nc.tensor.matmul(out=psd[:, 3 * t:3 * t + 3],
                             lhsT=oht[:, 128 * t:128 * (t + 1)],
                             rhs=sdw_all[:, 3 * wl:3 * wl + 3],
                             start=True, stop=True)
        e1 = spool.tile([128, 3 * K], F32, tag="e1")
        nc.vector.tensor_tensor(
            out=e1[:].rearrange("p (t c) -> p t c", t=K),
            in0=xg3[:, :, 128:131],
            in1=psd[:, 0:3 * K].rearrange("p (t c) -> p t c", t=K),
            op=mybir.AluOpType.add)
        sc = spool.tile([128, 3 * K], F32, tag="sc")
        nc.vector.tensor_scalar(out=sc[:], in0=e1[:], scalar1=NEG_SLOPE,
                                scalar2=None, op0=mybir.AluOpType.mult)
        t2 = spool.tile([128, 3 * K], F32, tag="t2")
        nc.vector.tensor_tensor(out=t2[:], in0=e1[:], in1=sc[:],
                                op=mybir.AluOpType.max)
        pq = spool.tile([128, 6 * K], BF16, tag="pq")
        q6 = pq[:].rearrange("p (t c) -> p t c", t=K)
        nc.scalar.activation(out=q6[:, :, 0:3],
                             in_=t2[:].rearrange("p (t c) -> p t c", t=K),
                             func=mybir.ActivationFunctionType.Exp)
        nc.vector.tensor_tensor(
            out=q6[:, :, 3:6], in0=q6[:, :, 0:3],
            in1=ewa[:].unsqueeze(2).broadcast_to([128, K, 3]),
            op=mybir.AluOpType.mult)

        # denominators: den[:,0:3]=sum p*onehot, den[:,3:6]=sum p*ew*onehot
        dn0 = 3 * K + 8
        den = psd[:, dn0:dn0 + 6]
        for t in range(K):
            nc.tensor.matmul(out=den, lhsT=ohw[:, 128 * t:128 * (t + 1)],
                             rhs=pq[:, 6 * t:6 * t + 6],
                             start=(t == 0), stop=(t == K - 1))
        t3 = fpool.tile([128, 3], F32, tag="t3")
        nc.vector.tensor_scalar(out=t3[:], in0=psd[:, dn0:dn0 + 3],
                                scalar1=1e-16, scalar2=3.0,
                                op0=mybir.AluOpType.max, op1=mybir.AluOpType.mult)
        inv3 = fpool.tile([128, 3], BF16, tag="inv3")
        with nc.allow_low_precision(reason="softmax denom reciprocal in bf16"):
            nc.vector.reciprocal(out=inv3[:], in_=t3[:])
        s1 = fpool.tile([128, 4], BF16, tag="s1")
        nc.vector.memset(s1[:, 3:4], 1.0)
        nc.vector.tensor_tensor(out=s1[:, 0:3], in0=psd[:, dn0 + 3:dn0 + 6],
                                in1=inv3[:], op=mybir.AluOpType.mult)
        # per-edge 1/(3*den): inv_e = ohT^T @ inv3
        for t in range(K):
            nc.tensor.matmul(out=psd[:, iv0 + 3 * t:iv0 + 3 * t + 3],
                             lhsT=oht[:, 128 * t:128 * (t + 1)], rhs=inv3[:],
                             start=True, stop=True)
        qa2f = spool.tile([128, 3 * K], F32, tag="qa2f")
        nc.vector.tensor_tensor(
            out=qa2f[:].rearrange("p (t c) -> p t c", t=K),
            in0=q6[:, :, 3:6],
            in1=psd[:, iv0:iv0 + 3 * K].rearrange("p (t c) -> p t c", t=K),
            op=mybir.AluOpType.mult)
        qa2 = spool.tile([128, 3 * K], BF16, tag="qa2")
        nc.vector.tensor_scalar(out=qa2[:], in0=qa2f[:], scalar1=1.0,
                                scalar2=None, op0=mybir.AluOpType.mult)

        # scatter matrices mst[h] = onehot * qa2_h, heads split across engines
        mst = mpool.tile([128, 3 * KC], BF16, tag="mst")
        m4 = mst[:].rearrange("p (h t n) -> p h t n", h=3, t=K)
        cc = 0
        while cc < K:
            cw = min(4, K - cc)
            nc.vector.tensor_tensor(
                out=m4[:, 0, cc:cc + cw, :], in0=o3[:, cc:cc + cw, :],
                in1=qa2[:, 3 * cc:3 * (cc + cw):3].unsqueeze(2)
                .broadcast_to([128, cw, 128]),
                op=mybir.AluOpType.mult)
            nc.gpsimd.tensor_tensor(
                out=m4[:, 2, cc:cc + cw, :], in0=o3[:, cc:cc + cw, :],
                in1=qa2[:, 3 * cc + 2:3 * (cc + cw):3].unsqueeze(2)
                .broadcast_to([128, cw, 128]),
                op=mybir.AluOpType.mult)
            cc += 4
        for t in range(K):
            nc.scalar.activation(out=m4[:, 1, t, :],
                                 in_=ohw[:, 128 * t:128 * (t + 1)],
                                 func=mybir.ActivationFunctionType.Copy,
                                 scale=qa2f[:, 3 * t + 1:3 * t + 2])

        # aggregate: agg[f, (h,n)] += x_src[f] * mst
        agg = ps_agg.tile([128, 3 * 128], F32, tag="agg")
        for t in range(K):
            nc.tensor.matmul(out=agg[:], lhsT=xg3[:, t, 0:128],
                             rhs=m4[:, :, t, :],
                             start=(t == 0), stop=(t == K - 1))

        # flush: out = sum_h agg_h@Wh/(3 den) + S1@(b@Wh) + mean bias
        aggsb = fpool.tile([128, 3 * 128], BF16, tag="aggsb")
        nc.scalar.activation(out=aggsb[:], in_=agg[:],
                             func=mybir.ActivationFunctionType.Copy)
        s1t_ps = psd[0:4, 256:384].bitcast(BF16)[:, 0:128]
        nc.tensor.transpose(out=s1t_ps, in_=s1[:], identity=ident_sb[:])
        s1t = fpool.tile([4, 128], BF16, tag="s1ts")
        nc.scalar.activation(out=s1t[:], in_=s1t_ps,
                             func=mybir.ActivationFunctionType.Copy)
        proj = ps_proj.tile([128, 128], F32, tag="proj")
        for h in range(3):
            nc.tensor.matmul(out=proj[:], lhsT=aggsb[:, 128 * h:128 * (h + 1)],
                             rhs=wh_sb[:, 128 * h:128 * (h + 1)],
                             start=(h == 0), stop=False)
        nc.tensor.matmul(out=proj[:], lhsT=s1t[:], rhs=bwh4_sb[:],
                         start=False, stop=True)
        out_sb = fpool.tile([128, 128], F32, tag="outsb")
        nc.scalar.activation(out=out_sb[:], in_=proj[:],
                             func=mybir.ActivationFunctionType.Copy)
        nc.sync.dma_start(out=outc[g0:g0 + 128, :], in_=out_sb[:])


def _build(K, NTA, NTB):
    nc = bacc.Bacc("TRN2", target_bir_lowering=False, debug=False,
                   num_devices=NCORES)
    xqA = nc.dram_tensor("xqA", [128, NTA * 128], BF16, kind="ExternalInput")
    xqB = nc.dram_tensor("xqB", [128, NTB * 128], BF16, kind="ExternalInput")
    wg = nc.dram_tensor("wg", [128, 134], BF16, kind="ExternalInput")
    biasr6 = nc.dram_tensor("biasr6", [128, 6], BF16, kind="ExternalInput")
    wh = nc.dram_tensor("wh", [128, 3 * 128], BF16, kind="ExternalInput")
    bwh4 = nc.dram_tensor("bwh4", [4, 128], BF16, kind="ExternalInput")
    iota_p = nc.dram_tensor("iota_p", [128, 1], BF16, kind="ExternalInput")
    iota_f = nc.dram_tensor("iota_f", [128, 128], BF16, kind="ExternalInput")
    ident = nc.dram_tensor("ident", [128, 128], BF16, kind="ExternalInput")
    earr = nc.dram_tensor("earr", [WPC * 128, 69 * K], I32, kind="ExternalInput")

    xtabA = nc.dram_tensor("xtabA", [NTA * 128, 256], BF16)
    xtabB = nc.dram_tensor("xtabB", [NTB * 128, 256], BF16)
    sds = nc.dram_tensor("sds", [WPC * 128, 3], BF16)
    outc = nc.dram_tensor("outc", [NPC, DIM], F32, kind="ExternalOutput")

    with TileContext(nc) as tc:
        with tc.tile_pool(name="const", bufs=1) as cpool:
            wg_sb = cpool.tile([128, 134], BF16, tag="wg")
            nc.sync.dma_start(out=wg_sb[:], in_=wg[:])
            biasr6_sb = cpool.tile([128, 6], BF16, tag="biasr6")
            nc.sync.dma_start(out=biasr6_sb[:], in_=biasr6[:])
            wh_sb = cpool.tile([128, 3 * 128], BF16, tag="wh")
            nc.sync.dma_start(out=wh_sb[:], in_=wh[:])
            bwh4_sb = cpool.tile([4, 128], BF16, tag="bwh4")
            nc.sync.dma_start(out=bwh4_sb[:], in_=bwh4[:])
            iota_p_sb = cpool.tile([128, 1], BF16, tag="iota_p")
            nc.sync.dma_start(out=iota_p_sb[:], in_=iota_p[:])
            iota_f_sb = cpool.tile([128, 128], BF16, tag="iota_f")
            nc.sync.dma_start(out=iota_f_sb[:], in_=iota_f[:])
            ident_sb = cpool.tile([128, 128], BF16, tag="ident")
            nc.sync.dma_start(out=ident_sb[:], in_=ident[:])

            _phase1(nc, tc, xqA, xtabA, sds, 0, NTA, WA, wg_sb, biasr6_sb)
            _phase1(nc, tc, xqB, xtabB, sds, WA * 128, NTB, WB, wg_sb, biasr6_sb)

            with (
                tc.tile_pool(name="win", bufs=4) as wpool,
                tc.tile_pool(name="gat", bufs=3) as gpool,
                tc.tile_pool(name="big", bufs=3) as bpool,
                tc.tile_pool(name="sml", bufs=4) as spool,
                tc.tile_pool(name="mst", bufs=3) as mpool,
                tc.tile_pool(name="fl", bufs=4) as fpool,
                tc.tile_pool(name="psS", bufs=3, space="PSUM") as ps_small,
                tc.tile_pool(name="psA", bufs=3, space="PSUM") as ps_agg,
                tc.tile_pool(name="psP", bufs=2, space="PSUM") as ps_proj,
            ):
                pools = (wpool, gpool, bpool, spool, mpool, fpool,
                         ps_small, ps_agg, ps_proj)
                _phase2_half(nc, tc, K, 0, WA, 0, xtabA, earr, sds, outc,
                             iota_p_sb, iota_f_sb, ident_sb, wh_sb, bwh4_sb,
                             pools)
                _phase2_half(nc, tc, K, 1, WB, WA, xtabB, earr, sds, outc,
                             iota_p_sb, iota_f_sb, ident_sb, wh_sb, bwh4_sb,
                             pools)

    nc.compile()
    return nc


def _prep(x, edge_index, edge_ids, ddi_weight, W_lin, b_lin, edge_emb,
          W_heads, att_src, att_dst, bias_heads):
    x = np.asarray(x, np.float32)
    src = np.asarray(edge_index[0]).astype(np.int64)
    dst = np.asarray(edge_index[1]).astype(np.int64)
    eids = np.asarray(edge_ids).astype(np.int64)
    ddi = np.asarray(ddi_weight, np.float32)
    W_lin = np.asarray(W_lin, np.float32)
    b_lin = np.asarray(b_lin, np.float32)
    edge_emb = np.asarray(edge_emb, np.float32)
    W_heads = np.asarray(W_heads, np.float32)
    att_src = np.asarray(att_src, np.float32)
    att_dst = np.asarray(att_dst, np.float32)
    bias_heads = np.asarray(bias_heads, np.float32)
    ew = edge_emb[eids, 0] - ddi

    # --- balance nodes into NWIN windows of 128 nodes, equal edge counts ---
    import heapq
    deg = np.bincount(dst, minlength=N_NODES)
    order = np.argsort(-deg, kind="stable")
    heap = [(0, w) for w in range(NWIN)]
    heapq.heapify(heap)
    slots_used = np.zeros(NWIN, np.int32)
    loads = np.zeros(NWIN, np.int64)
    win_of = np.empty(N_NODES, np.int32)
    slot_of = np.empty(N_NODES, np.int32)
    for n in order:
        load, w = heapq.heappop(heap)
        win_of[n] = w
        slot_of[n] = slots_used[w]
        slots_used[w] += 1
        loads[w] += deg[n]
        if slots_used[w] < NPW:
            heapq.heappush(heap, (int(loads[w]), w))
    K = int((loads.max() + NPW - 1) // NPW)
    K += K % 2  # even, for bf16 pairs in the i32 container

    ewin = win_of[dst]                # window of each edge
    eorder = np.argsort(ewin, kind="stable")
    esrc = src[eorder]
    edst = dst[eorder]
    eew = ew[eorder]
    ewin_s = ewin[eorder]
    wbounds = np.searchsorted(ewin_s, np.arange(NWIN + 1))

    # --- per-core compact tables and edge arrays ---
    halves = [(0, WA), (WA, WB)]
    NTA = NTB = 0
    core_data = []
    for c in range(NCORES):
        hd = []
        for hi, (w0, nw) in enumerate(halves):
            gw0 = c * WPC + w0
            own_nodes = np.full(nw * 128, -1, np.int64)
            for wl in range(nw):
                wsel = np.where(win_of == gw0 + wl)[0]
                own_nodes[wl * 128 + slot_of[wsel]] = wsel
            e0, e1 = wbounds[gw0], wbounds[gw0 + nw]
            hsrc = esrc[e0:e1]
            own_set = own_nodes[own_nodes >= 0]
            relab = np.full(N_NODES, -1, np.int32)
            own_rows = np.where(own_nodes >= 0)[0]
            relab[own_nodes[own_rows]] = own_rows
            extra = np.unique(hsrc)
            extra = extra[relab[extra] < 0]
            base = nw * 128
            relab[extra] = base + np.arange(len(extra), dtype=np.int32)
            nrows = base + len(extra)
            assert nrows <= 32767, f"compact table too large: {nrows}"
            node_of_row = np.full(nrows, -1, np.int64)
            node_of_row[own_rows] = own_nodes[own_rows]
            node_of_row[base:] = extra
            hd.append(dict(w0=w0, nw=nw, gw0=gw0, e0=e0, e1=e1,
                           relab=relab, node_of_row=node_of_row, nrows=nrows))
        NTA = max(NTA, (hd[0]["nrows"] + 127) // 128)
        NTB = max(NTB, (hd[1]["nrows"] + 127) // 128)
        core_data.append(hd)

    # --- weights / consts ---
    asd = np.zeros((DIM, 6), np.float32)
    for h in range(N_HEADS):
        asd[:, h] = W_heads[h] @ att_src[h]
        asd[:, 3 + h] = W_heads[h] @ att_dst[h]
    wg = np.zeros((DIM, 134), np.float32)
    wg[:, 0:128] = W_lin
    wg[:, 128:134] = W_lin @ asd
    biasr6 = np.tile(b_lin @ asd, (128, 1))
    wh2 = np.zeros((128, 3 * 128), np.float32)
    for h in range(N_HEADS):
        wh2[:, h * 128:(h + 1) * 128] = W_heads[h]
    bwh4 = np.zeros((4, 128), np.float32)
    for h in range(N_HEADS):
        bwh4[h] = b_lin @ W_heads[h]
    bwh4[3] = bias_heads.mean(0)
    iota_p = np.arange(128, dtype=np.float32).reshape(128, 1)
    iota_f = np.tile(np.arange(128, dtype=np.float32), (128, 1))
    ident = np.eye(128, dtype=np.float32)
    shared = dict(wg=wg.astype(BF), biasr6=biasr6.astype(BF),
                  wh=wh2.astype(BF), bwh4=bwh4.astype(BF),
                  iota_p=iota_p.astype(BF), iota_f=iota_f.astype(BF),
                  ident=ident.astype(BF))

    in_maps = []
    for c in range(NCORES):
        m = dict(shared)
        earr = np.zeros((WPC * 128, 69 * K), np.int32)
        for hi, (w0, nw) in enumerate(halves):
            hdd = core_data[c][hi]
            nt = NTA if hi == 0 else NTB
            xq = np.zeros((128, nt * 128), BF)
            valid = hdd["node_of_row"] >= 0
            cols = np.where(valid)[0]
            xq[:, cols] = x[hdd["node_of_row"][cols]].T.astype(BF)
            m["xqA" if hi == 0 else "xqB"] = xq
            relab = hdd["relab"]
            for wl in range(nw):
                gw = hdd["gw0"] + wl
                e0, e1 = wbounds[gw], wbounds[gw + 1]
                mcount = e1 - e0
                # per-window K*128 edge slots, j -> (p=j%128, t=j//128)
                gi = np.zeros(K * 128, np.int16)
                gi[:mcount] = relab[esrc[e0:e1]].astype(np.int16)
                dc = np.full(K * 128, 128.0, np.float32)
                dc[:mcount] = slot_of[edst[e0:e1]].astype(np.float32)
                ewv = np.zeros(K * 128, np.float32)
                ewv[:mcount] = eew[e0:e1]
                g16 = np.zeros((16, K * 8), np.int16)
                g16[np.arange(K * 128) % 16, np.arange(K * 128) // 16] = gi
                row = (w0 + wl) * 128
                blk = earr[row:row + 128]
                blk[:, 0:4 * K] = np.tile(g16, (8, 1)).view(np.int32)
                dcol = np.ascontiguousarray(dc.reshape(K, 128).T).astype(BF)
                ecol = np.ascontiguousarray(ewv.reshape(K, 128).T).astype(BF)
                blk[:, 4 * K:4 * K + K // 2] = dcol.view(np.int32)
                blk[:, 4 * K + K // 2:5 * K] = ecol.view(np.int32)
                dstb = np.tile(dc.astype(BF), (128, 1))     # [128, K*128]
                blk[:, 5 * K:69 * K] = dstb.view(np.int32)
        m["earr"] = earr
        in_maps.append(m)

    key = (K, NTA, NTB)
    # map node -> global output row
    gslot = (win_of.astype(np.int64) // WPC) * NPC + \
        (win_of.astype(np.int64) % WPC) * 128 + slot_of
    return key, dict(in_maps=in_maps, gslot=gslot)


def kernel(**inputs):
    key, d = _prep(**inputs)
    if key not in _cache:
        _cache[key] = _build(*key)
    nc = _cache[key]
    res = bass_utils.run_bass_kernel_spmd(nc, d["in_maps"],
                                          core_ids=list(range(NCORES)))
    big = np.concatenate([res.results[c]["outc"] for c in range(NCORES)],
                         axis=0)
    out = big[d["gslot"]]
    return np.ascontiguousarray(out).astype(np.float32)
